# revision 1
# baseline (speedup 1.0000x reference)
"""Trainium2 Bass kernel for spatial attention (GroupNorm + QKV + softmax attention
+ output projection + residual), distributed over 8 NeuronCores.

Sharding: core = 2*b + hp handles image b (of 4) and head pair hp (heads 2hp, 2hp+1).
Each core computes GroupNorm(x[b]), its heads' q/k/v, full spatial attention for its
two heads, and a partial output projection (out_w columns for its heads). Core pairs'
partial outputs are summed on the host (hp==0 core carries the residual + bias).
"""

import numpy as np

import concourse.bass as bass
import concourse.bacc as bacc
import concourse.tile as tile
from concourse import mybir
from concourse import bass_utils
from concourse.alu_op_type import AluOpType

B, C, H, W = 4, 256, 48, 48
HW = H * W  # 2304
NH, HD = 4, 64
G, GC = 16, 16  # 16 groups x 16 channels
EPS = 1e-5
NCORES = 8
JC = 128  # j (key spatial) chunk
NJ = HW // JC  # 18
IBLKS = [(0, 512), (512, 1024), (1024, 1536), (1536, 2048), (2048, 2304)]
HALF = HW // 2  # 1152

F32 = mybir.dt.float32
F32R = mybir.dt.float32r
BF16 = mybir.dt.bfloat16
AX = mybir.AxisListType.X
AF = mybir.ActivationFunctionType
OP = AluOpType


def _nchunks(size, step=512):
    # PSUM-bank-aligned chunks: a matmul output may not cross a 512-fp32 bank boundary
    return [(a, min(a + step, size)) for a in range(0, size, step)]


def _build(mm_dt=F32R):
    nc = bacc.Bacc("TRN2", target_bir_lowering=False, debug=False, enable_asserts=False)

    def cast(ap):
        return ap

    x_d = nc.dram_tensor("x", [C, HW], F32, kind="ExternalInput").ap()
    res_d = nc.dram_tensor("res", [C, HW], F32, kind="ExternalInput").ap()
    wq_d = nc.dram_tensor("wq", [C, 2 * HD], F32, kind="ExternalInput").ap()
    wk_d = nc.dram_tensor("wk", [C, 2 * HD], F32, kind="ExternalInput").ap()
    wv_d = nc.dram_tensor("wv", [C, 2 * HD], F32, kind="ExternalInput").ap()
    wo_d = nc.dram_tensor("wo", [2 * HD, C], F32, kind="ExternalInput").ap()
    gnp_d = nc.dram_tensor("gnp", [C, 2], F32, kind="ExternalInput").ap()
    gind_d = nc.dram_tensor("gind", [128, 32], F32, kind="ExternalInput").ap()
    gbc_d = nc.dram_tensor("gbc", [16, C], F32, kind="ExternalInput").ap()
    y_d = nc.dram_tensor("y", [C, HW], F32, kind="ExternalOutput").ap()

    with tile.TileContext(nc) as tc:
        with (
            tc.tile_pool(name="consts", bufs=1) as consts,
            tc.tile_pool(name="big", bufs=1) as big,
            tc.tile_pool(name="small", bufs=4) as small,
            tc.tile_pool(name="pt", bufs=4) as ptp,
        ):
            # ---- constant / weight loads ----
            gind_sb = consts.tile([128, 32], F32, tag="gind", name="gind")
            nc.sync.dma_start(gind_sb[:], gind_d[:])
            gbc_sb = consts.tile([16, C], F32, tag="gbc", name="gbc")
            nc.sync.dma_start(gbc_sb[:], gbc_d[:])
            w_sb = {}
            for name, d in (("wq", wq_d), ("wk", wk_d), ("wv", wv_d)):
                for kc in range(2):
                    tf = consts.tile([128, 2 * HD], F32, tag=f"{name}{kc}f", name=f"{name}{kc}f")
                    nc.sync.dma_start(tf[:], d[kc * 128 : (kc + 1) * 128, :])
                    t = consts.tile([128, 2 * HD], mm_dt, tag=f"{name}{kc}", name=f"{name}{kc}")
                    nc.vector.tensor_copy(t[:], tf[:])
                    w_sb[name, kc] = t
            wof = consts.tile([128, C], F32, tag="wof", name="wof")
            nc.sync.dma_start(wof[:], wo_d[:])
            wo_sb = consts.tile([128, C], mm_dt, tag="wo", name="wo")
            nc.vector.tensor_copy(wo_sb[:], wof[:])
            gnp_sb = []
            for ct in range(2):
                t = consts.tile([128, 2], F32, tag=f"gnp{ct}", name=f"gnp{ct}")
                nc.sync.dma_start(t[:], gnp_d[ct * 128 : (ct + 1) * 128, :])
                gnp_sb.append(t)

            x_sb, xn_sb, res_sb = [], [], []
            for ct in range(2):
                t = big.tile([128, HW], F32, tag=f"x{ct}", name=f"x{ct}")
                nc.sync.dma_start(t[:], x_d[ct * 128 : (ct + 1) * 128, :])
                x_sb.append(t)
                xn_sb.append(big.tile([128, HW], mm_dt, tag=f"xn{ct}", name=f"xn{ct}"))
                r = big.tile([128, HW], F32, tag=f"res{ct}", name=f"res{ct}")
                nc.sync.dma_start(r[:], res_d[ct * 128 : (ct + 1) * 128, :])
                res_sb.append(r)

            # ---- GroupNorm ----
            # per-channel sums / sum-of-squares -> stats[:, (s0,q0,s1,q1)]
            stats = small.tile([128, 4], F32, tag="stats", name="stats")
            for ct in range(2):
                nc.vector.reduce_sum(stats[:, 2 * ct : 2 * ct + 1], x_sb[ct][:], axis=AX)
                nc.vector.scalar_tensor_tensor(
                    xn_sb[ct][:], x_sb[ct][:], 1.0, x_sb[ct][:],
                    op0=OP.mult, op1=OP.mult,
                    accum_out=stats[:, 2 * ct + 1 : 2 * ct + 2],
                )
            with tc.tile_pool(name="ps_gn", bufs=2, space=bass.MemorySpace.PSUM) as ps_gn:
                # accumulate both channel tiles' per-group (sum, sumsq) into [16, 2]
                g_ps = ps_gn.tile([16, 2], F32, tag="g", name="g")
                nc.tensor.matmul(g_ps[:], gind_sb[:, 0:16], stats[:, 0:2], start=True, stop=False)
                nc.tensor.matmul(g_ps[:], gind_sb[:, 16:32], stats[:, 2:4], start=False, stop=True)
                mall = small.tile([16, 2], F32, tag="mall", name="mall")
                nc.vector.tensor_scalar_mul(mall[:], g_ps[:], 1.0 / (GC * HW))
                msq = small.tile([16, 1], F32, tag="msq", name="msq")
                nc.vector.tensor_tensor(msq[:], mall[:, 0:1], mall[:, 0:1], op=OP.mult)
                ve = small.tile([16, 1], F32, tag="ve", name="ve")
                nc.vector.tensor_tensor(ve[:], mall[:, 1:2], msq[:], op=OP.subtract)
                ve2 = small.tile([16, 1], F32, tag="ve2", name="ve2")
                nc.vector.tensor_scalar_add(ve2[:], ve[:], EPS)
                sq = small.tile([16, 1], F32, tag="sq", name="sq")
                nc.scalar.activation(sq[:], ve2[:], AF.Sqrt)
                r0 = small.tile([16, 1], F32, tag="r0", name="r0")
                nc.vector.reciprocal(r0[:], sq[:])
                # sqrt LUT is loose; one Newton step: r = r0 * (1.5 - 0.5 * v * r0^2)
                t1 = small.tile([16, 1], F32, tag="t1", name="t1")
                nc.vector.tensor_tensor(t1[:], r0[:], r0[:], op=OP.mult)
                t2 = small.tile([16, 1], F32, tag="t2", name="t2")
                nc.vector.tensor_tensor(t2[:], ve2[:], t1[:], op=OP.mult)
                t3 = small.tile([16, 1], F32, tag="t3", name="t3")
                nc.vector.tensor_scalar(t3[:], t2[:], -0.5, 1.5, op0=OP.mult, op1=OP.add)
                # gvals [16, 2] = per-group (mean, rstd)
                gvals = small.tile([16, 2], F32, tag="gvals", name="gvals")
                nc.vector.tensor_copy(gvals[:, 0:1], mall[:, 0:1])
                nc.vector.tensor_tensor(gvals[:, 1:2], r0[:], t3[:], op=OP.mult)
                for ct in range(2):
                    cv = ps_gn.tile([128, 2], F32, tag="cv", name="cv")
                    nc.tensor.matmul(
                        cv[:], gbc_sb[:, ct * 128 : (ct + 1) * 128], gvals[:],
                        start=True, stop=True,
                    )
                    scale_t = small.tile([128, 1], F32, tag="scale", name="scale")
                    nc.vector.tensor_tensor(scale_t[:], gnp_sb[ct][:, 0:1], cv[:, 1:2], op=OP.mult)
                    tb = small.tile([128, 1], F32, tag="tb", name="tb")
                    nc.vector.tensor_tensor(tb[:], cv[:, 0:1], scale_t[:], op=OP.mult)
                    bias_t = small.tile([128, 1], F32, tag="bias", name="bias")
                    nc.vector.tensor_tensor(bias_t[:], gnp_sb[ct][:, 1:2], tb[:], op=OP.subtract)
                    nc.vector.tensor_scalar(
                        xn_sb[ct][:], x_sb[ct][:], scale_t[:], bias_t[:],
                        op0=OP.mult, op1=OP.add,
                    )

            # ---- QKV projections ----
            q_sb = big.tile([128, HW], mm_dt, tag="q", name="q")
            k_sb = big.tile([128, HW], mm_dt, tag="k", name="k")
            vt_sb = []
            for h in range(2):
                t = big.tile([128, NJ * (HD + 1)], mm_dt, tag=f"vt{h}", name=f"vt{h}")
                t3 = t[:].rearrange("p (j c) -> p j c", c=HD + 1)
                # fill the per-chunk "ones" column (denominator accumulator rows);
                # memset can't write f32r, so use (x*0 + 1) via tensor_scalar
                nc.vector.tensor_scalar(
                    t3[:, :, HD : HD + 1], x_sb[0][:, 0:NJ], 0.0, 1.0,
                    op0=OP.mult, op1=OP.add,
                )
                vt_sb.append(t)

            with tc.tile_pool(name="ps_qkv", bufs=2, space=bass.MemorySpace.PSUM) as ps_qkv:
                # v^T first: its DVE copies then overlap the q/k matmuls, and
                # q/k (which attention needs first) are ready right at the boundary
                for half in range(2):
                    vps = ps_qkv.tile([128, HALF], F32, tag="qkv", name="qkv")
                    for j9 in range(9):
                        jc = half * 9 + j9
                        for kc in range(2):
                            nc.tensor.matmul(
                                vps[:, j9 * 128 : (j9 + 1) * 128],
                                xn_sb[kc][:, jc * JC : (jc + 1) * JC],
                                w_sb["wv", kc][:],
                                start=(kc == 0), stop=(kc == 1),
                            )
                    vps3 = vps[:].rearrange("p (j c) -> p j c", c=128)
                    for h in range(2):
                        dst3 = vt_sb[h][:].rearrange("p (j c) -> p j c", c=HD + 1)
                        nc.vector.tensor_copy(
                            dst3[:, half * 9 : (half + 1) * 9, 0:HD],
                            vps3[:, :, h * HD : (h + 1) * HD],
                        )
                for dst, wname in ((q_sb, "wq"), (k_sb, "wk")):
                    for half in range(2):
                        ps = ps_qkv.tile([128, HALF], F32, tag="qkv", name="qkv")
                        for kc in range(2):
                            for n0, n1 in _nchunks(HALF):
                                nc.tensor.matmul(
                                    ps[:, n0:n1],
                                    cast(w_sb[wname, kc][:]),
                                    cast(xn_sb[kc][:, half * HALF + n0 : half * HALF + n1]),
                                    start=(kc == 0), stop=(kc == 1),
                                )
                        nc.vector.tensor_copy(dst[:, half * HALF : (half + 1) * HALF], ps[:])

            # ---- attention ----
            headout = big.tile([128, HW], mm_dt, tag="headout", name="headout")
            with tc.tile_pool(name="ps_att", bufs=1, space=bass.MemorySpace.PSUM) as ps_att:
                for i0, i1 in IBLKS:
                    blk = i1 - i0
                    # h0/h1 S^T outputs must land in DIFFERENT psum banks: concurrent
                    # row-tiled matmuls writing the same bank crash the device.
                    salign = ((blk + 511) // 512) * 512
                    u = [ps_att.tile([HD + 1, blk], F32, tag=f"u{h}", name=f"u{h}", bufs=2) for h in range(2)]
                    def emit_s(jc):
                        st = ps_att.tile([128, 2 * salign], F32, tag="s", name="s", bufs=2)
                        for h in range(2):
                            lhsT = k_sb[h * HD : (h + 1) * HD, jc * JC : (jc + 1) * JC]
                            for n0, n1 in _nchunks(blk, 512):
                                nc.tensor.matmul(
                                    st[:, h * salign + n0 : h * salign + n1],
                                    cast(lhsT),
                                    cast(q_sb[h * HD : (h + 1) * HD, i0 + n0 : i0 + n1]),
                                    start=True, stop=True,
                                )
                        pt = ptp.tile([128, 2 * blk], mm_dt, tag="pt", name="pt")
                        if blk == salign:
                            nc.scalar.activation(pt[:], st[:], AF.Exp, scale=1.0 / 16.0)
                        else:
                            for h in range(2):
                                nc.scalar.activation(
                                    pt[:, h * blk : (h + 1) * blk],
                                    st[:, h * salign : h * salign + blk],
                                    AF.Exp, scale=1.0 / 16.0,
                                )
                        return pt

                    def emit_pv(jc, pt):
                        for h in range(2):
                            lhsT = vt_sb[h][:, jc * (HD + 1) : (jc + 1) * (HD + 1)]
                            for n0, n1 in _nchunks(blk, 512):
                                nc.tensor.matmul(
                                    u[h][:, n0:n1],
                                    cast(lhsT),
                                    cast(pt[:, h * blk + n0 : h * blk + n1]),
                                    start=(jc == 0), stop=(jc == NJ - 1),
                                )

                    # software-pipeline by one stage: PE computes S(jc+1) while
                    # ACT exponentiates S(jc), so the PE stream never blocks on exp
                    prev_pt = emit_s(0)
                    for jc in range(1, NJ):
                        pt = emit_s(jc)
                        emit_pv(jc - 1, prev_pt)
                        prev_pt = pt
                    emit_pv(NJ - 1, prev_pt)
                    for h in range(2):
                        dn = small.tile([1, blk], F32, tag="dn", name="dn")
                        nc.vector.tensor_copy(dn[:], u[h][HD : HD + 1, :])
                        rcp = small.tile([1, blk], F32, tag="rcp", name="rcp")
                        scr = small.tile([1, blk], F32, tag="scr", name="scr")
                        nc.vector.reciprocal_approx_accurate(rcp[:], dn[:], scr[:])
                        rb = small.tile([HD, blk], F32, tag="rb", name="rb")
                        nc.gpsimd.partition_broadcast(rb[:], rcp[:])
                        nc.vector.tensor_tensor(
                            headout[h * HD : (h + 1) * HD, i0:i1],
                            u[h][0:HD, :], rb[:], op=OP.mult,
                        )

            # ---- output projection + residual ----
            with tc.tile_pool(name="ps_out", bufs=2, space=bass.MemorySpace.PSUM) as ps_out:
                for mt in range(2):
                    for half in range(2):
                        yp = ps_out.tile([128, HALF], F32, tag="yp", name="yp")
                        for n0, n1 in _nchunks(HALF):
                            nc.tensor.matmul(
                                yp[:, n0:n1],
                                cast(wo_sb[:, mt * 128 : (mt + 1) * 128]),
                                cast(headout[:, half * HALF + n0 : half * HALF + n1]),
                                start=True, stop=True,
                            )
                        yo = small.tile([128, HALF], F32, tag="yo", name="yo")
                        nc.vector.tensor_tensor(
                            yo[:], yp[:],
                            res_sb[mt][:, half * HALF : (half + 1) * HALF], op=OP.add,
                        )
                        nc.sync.dma_start(
                            y_d[mt * 128 : (mt + 1) * 128, half * HALF : (half + 1) * HALF],
                            yo[:],
                        )

    nc.compile()
    return nc


def _consts():
    # gind[:, 0:16]: tile-0 channel -> group one-hot; [:, 16:32]: tile-1 channel -> group
    gind = np.zeros((128, 32), np.float32)
    for c in range(128):
        gind[c, c // GC] = 1.0
        gind[c, 16 + 8 + c // GC] = 1.0
    gbc = np.zeros((16, C), np.float32)
    for c in range(C):
        gbc[c // GC, c] = 1.0
    return gind, gbc


def make_in_maps(x, gn_weight, gn_bias, qkv_w, out_w, out_b):
    x = np.asarray(x, np.float32)
    qkv_w = np.asarray(qkv_w, np.float32)
    out_w = np.asarray(out_w, np.float32)
    out_b = np.asarray(out_b, np.float32)
    gn_weight = np.asarray(gn_weight, np.float32)
    gn_bias = np.asarray(gn_bias, np.float32)
    xr = np.ascontiguousarray(x.reshape(B, C, HW))
    gind, gbc = _consts()
    gnp = np.ascontiguousarray(np.stack([gn_weight, gn_bias], axis=1))
    in_maps = []
    for core in range(NCORES):
        b, hp = divmod(core, 2)
        heads = (2 * hp, 2 * hp + 1)
        qs = np.concatenate([qkv_w[n * 192 : n * 192 + 64] for n in heads], 0)
        ks = np.concatenate([qkv_w[n * 192 + 64 : n * 192 + 128] for n in heads], 0)
        vs = np.concatenate([qkv_w[n * 192 + 128 : n * 192 + 192] for n in heads], 0)
        res = xr[b] + out_b[:, None] if hp == 0 else np.zeros_like(xr[b])
        in_maps.append({
            "x": xr[b],
            "res": np.ascontiguousarray(res, np.float32),
            "wq": np.ascontiguousarray(qs.T),
            "wk": np.ascontiguousarray(ks.T),
            "wv": np.ascontiguousarray(vs.T),
            "wo": np.ascontiguousarray(out_w[:, hp * 128 : (hp + 1) * 128].T),
            "gnp": gnp,
            "gind": gind,
            "gbc": gbc,
        })
    return in_maps


_NC_CACHE = {}


def get_nc(mm_dt=F32R):
    key = str(mm_dt)
    if key not in _NC_CACHE:
        _NC_CACHE[key] = _build(mm_dt)
    return _NC_CACHE[key]


def kernel(x, gn_weight, gn_bias, qkv_w, out_w, out_b):
    nc = get_nc(BF16)
    in_maps = make_in_maps(x, gn_weight, gn_bias, qkv_w, out_w, out_b)
    res = bass_utils.run_bass_kernel_spmd(nc, in_maps, core_ids=list(range(NCORES)))
    y = np.empty((B, C, HW), np.float32)
    for b in range(B):
        y[b] = res.results[2 * b]["y"] + res.results[2 * b + 1]["y"]
    return y.reshape(B, C, H, W)



# revision 6
# speedup vs baseline: 1.2909x; 1.2909x over previous
"""Trainium2 Bass kernel for spatial attention (GroupNorm + QKV + softmax attention
+ output projection + residual), distributed over 8 NeuronCores.

Sharding: core = 2*b + hp handles image b (of 4) and head pair hp (heads 2hp, 2hp+1).
Each core computes GroupNorm(x[b]), its heads' q/k/v, full spatial attention for its
two heads, and a partial output projection (out_w columns for its heads). Core pairs'
partial outputs are summed on the host (hp==0 core carries the residual + bias).

Engine plan: exp for head 0 runs on the Scalar (ACT) engine (exp -> fp8 direct);
exp for head 1 runs on the Vector engine as a Schraudolph bit-trick
(int8(x*A+B) bitcast as fp8e4m3). P@V runs as fp8 DoubleRow matmuls (two key
chunks per pass). Attention output projection + residual + store happen per
query block so the tail is short.
"""

import numpy as np

import concourse.bass as bass
import concourse.bacc as bacc
import concourse.tile as tile
from concourse import mybir
from concourse import bass_utils
from concourse.alu_op_type import AluOpType

B, C, H, W = 4, 256, 48, 48
HW = H * W  # 2304
NH, HD = 4, 64
G, GC = 16, 16  # 16 groups x 16 channels
EPS = 1e-5
NCORES = 8
JC = 128  # key chunk
NJ = HW // JC  # 18
NT = NJ // 2  # 9 double-chunks for DoubleRow PV
IBLKS = [(0, 512), (512, 1024), (1024, 1536), (1536, 2048), (2048, 2304)]
HALF = HW // 2  # 1152
MPAD = 80  # vt pair stride (>=65, multiple of 16 for DoubleRow ldweights)

F32 = mybir.dt.float32
BF16 = mybir.dt.bfloat16
F8 = mybir.dt.float8e4
I8 = mybir.dt.int8
I32 = mybir.dt.int32
AX = mybir.AxisListType.X
AF = mybir.ActivationFunctionType
OP = AluOpType
DR = mybir.MatmulPerfMode.DoubleRow

LOG2E = 1.4426950408889634
A8 = 8.0 * LOG2E / 16.0        # schraudolph slope (1/16 softmax scale folded in)
B8 = 7.0 * 8.0 - 0.344         # e4m3 exponent bias, centered interp correction
RSQRT_K = 1597463007.0         # 0x5f3759df


def _nchunks(size, step=512):
    # PSUM-bank-aligned chunks: a matmul output may not cross a 512-fp32 bank boundary
    return [(a, min(a + step, size)) for a in range(0, size, step)]


def _build():
    nc = bacc.Bacc("TRN2", target_bir_lowering=False, debug=False, enable_asserts=False)

    x_d = nc.dram_tensor("x", [C, HW], F32, kind="ExternalInput").ap()
    res_d = nc.dram_tensor("res", [C, HW], F32, kind="ExternalInput").ap()
    wq_d = nc.dram_tensor("wq", [C, 2 * HD], F32, kind="ExternalInput").ap()
    wk_d = nc.dram_tensor("wk", [C, 2 * HD], F32, kind="ExternalInput").ap()
    wv_d = nc.dram_tensor("wv", [C, 2 * HD], F32, kind="ExternalInput").ap()
    wo_d = nc.dram_tensor("wo", [2 * HD, C], F32, kind="ExternalInput").ap()
    gnp_d = nc.dram_tensor("gnp", [C, 2], F32, kind="ExternalInput").ap()
    gind_d = nc.dram_tensor("gind", [128, 32], F32, kind="ExternalInput").ap()
    gbc_d = nc.dram_tensor("gbc", [16, C], F32, kind="ExternalInput").ap()
    y_d = nc.dram_tensor("y", [C, HW], F32, kind="ExternalOutput").ap()

    with tile.TileContext(nc) as tc:
        with (
            tc.tile_pool(name="consts", bufs=1) as consts,
            tc.tile_pool(name="big", bufs=1) as big,
            tc.tile_pool(name="small", bufs=4) as small,
            tc.tile_pool(name="pt", bufs=4) as ptp,
        ):
            # ---- input / weight loads (x first: GN needs it; res is only
            # needed at the per-block output stage, so it queues last) ----
            x_sb, xn_sb = [], []
            for ct in range(2):
                t = big.tile([128, HW], F32, tag=f"x{ct}", name=f"x{ct}")
                for h2 in range(2):
                    nc.sync.dma_start(
                        t[:, h2 * HALF : (h2 + 1) * HALF],
                        x_d[ct * 128 : (ct + 1) * 128, h2 * HALF : (h2 + 1) * HALF],
                    )
                x_sb.append(t)
                xn_sb.append(big.tile([128, HW], BF16, tag=f"xn{ct}", name=f"xn{ct}"))
            gind_sb = consts.tile([128, 32], F32, tag="gind", name="gind")
            nc.sync.dma_start(gind_sb[:], gind_d[:])
            gbc_sb = consts.tile([16, C], F32, tag="gbc", name="gbc")
            nc.sync.dma_start(gbc_sb[:], gbc_d[:])
            gnp_sb = []
            for ct in range(2):
                t = consts.tile([128, 2], F32, tag=f"gnp{ct}", name=f"gnp{ct}")
                nc.sync.dma_start(t[:], gnp_d[ct * 128 : (ct + 1) * 128, :])
                gnp_sb.append(t)
            w_sb = {}
            for name, d in (("wq", wq_d), ("wk", wk_d), ("wv", wv_d)):
                for kc in range(2):
                    tf = consts.tile([128, 2 * HD], F32, tag=f"{name}{kc}f", name=f"{name}{kc}f")
                    nc.sync.dma_start(tf[:], d[kc * 128 : (kc + 1) * 128, :])
                    t = consts.tile([128, 2 * HD], BF16, tag=f"{name}{kc}", name=f"{name}{kc}")
                    nc.vector.tensor_copy(t[:], tf[:])
                    w_sb[name, kc] = t
            wof = consts.tile([128, C], F32, tag="wof", name="wof")
            nc.sync.dma_start(wof[:], wo_d[:])
            wo_sb = consts.tile([128, C], BF16, tag="wo", name="wo")
            nc.vector.tensor_copy(wo_sb[:], wof[:])
            res_sb = []
            for ct in range(2):
                r = big.tile([128, HW], F32, tag=f"res{ct}", name=f"res{ct}")
                nc.sync.dma_start(r[:], res_d[ct * 128 : (ct + 1) * 128, :])
                res_sb.append(r)

            # preload the exp ACT table while DMAs run (first ACT instruction
            # in program order pulls in the exp_and_others set)
            scr_exp = small.tile([1, 2], F32, tag="screxp", name="screxp")
            nc.scalar.activation(scr_exp[:], gind_sb[0:1, 0:2], AF.Exp)

            # ---- GroupNorm stats (per x half-tile, so work starts as DMA lands) ----
            stats = small.tile([128, 8], F32, tag="stats", name="stats")
            for ct in range(2):
                for h2 in range(2):
                    i = 2 * ct + h2
                    sl = x_sb[ct][:, h2 * HALF : (h2 + 1) * HALF]
                    nc.vector.reduce_sum(stats[:, 2 * i : 2 * i + 1], sl, axis=AX)
                    nc.vector.scalar_tensor_tensor(
                        xn_sb[ct][:, h2 * HALF : (h2 + 1) * HALF], sl, 1.0, sl,
                        op0=OP.mult, op1=OP.mult,
                        accum_out=stats[:, 2 * i + 1 : 2 * i + 2],
                    )
            with tc.tile_pool(name="ps_gn", bufs=2, space=bass.MemorySpace.PSUM) as ps_gn:
                g_ps = ps_gn.tile([16, 2], F32, tag="g", name="g")
                for ct in range(2):
                    for h2 in range(2):
                        i = 2 * ct + h2
                        nc.tensor.matmul(
                            g_ps[:], gind_sb[:, ct * 16 : ct * 16 + 16],
                            stats[:, 2 * i : 2 * i + 2],
                            start=(i == 0), stop=(i == 3),
                        )
                mall = small.tile([16, 2], F32, tag="mall", name="mall")
                nc.vector.tensor_scalar_mul(mall[:], g_ps[:], 1.0 / (GC * HW))
                msq = small.tile([16, 1], F32, tag="msq", name="msq")
                nc.vector.tensor_tensor(msq[:], mall[:, 0:1], mall[:, 0:1], op=OP.mult)
                ve = small.tile([16, 1], F32, tag="ve", name="ve")
                nc.vector.tensor_tensor(ve[:], mall[:, 1:2], msq[:], op=OP.subtract)
                ve2 = small.tile([16, 1], F32, tag="ve2", name="ve2")
                nc.vector.tensor_scalar_add(ve2[:], ve[:], EPS)
                # rsqrt bit-trick seed (DVE only; avoids ACT sqrt table load):
                # y0 = bitcast_f32(int32(K - 0.5 * bits(v)))
                r0i = small.tile([16, 1], I32, tag="r0i", name="r0i")
                nc.vector.tensor_scalar(
                    r0i[:], ve2[:].bitcast(I32), -0.5, RSQRT_K, op0=OP.mult, op1=OP.add
                )
                # two Newton steps: y = y0 * (1.5 - 0.5 * v * y0^2)
                cur = r0i[:].bitcast(F32)
                for it in range(2):
                    ysq = small.tile([16, 1], F32, tag=f"ysq{it}", name=f"ysq{it}")
                    nc.vector.tensor_tensor(ysq[:], cur, cur, op=OP.mult)
                    hv = small.tile([16, 1], F32, tag=f"hv{it}", name=f"hv{it}")
                    nc.vector.scalar_tensor_tensor(
                        hv[:], ve2[:], -0.5, ysq[:], op0=OP.mult, op1=OP.mult
                    )
                    hv2 = small.tile([16, 1], F32, tag=f"hv2{it}", name=f"hv2{it}")
                    nc.vector.tensor_scalar_add(hv2[:], hv[:], 1.5)
                    yn = small.tile([16, 1], F32, tag=f"yn{it}", name=f"yn{it}")
                    nc.vector.tensor_tensor(yn[:], cur, hv2[:], op=OP.mult)
                    cur = yn[:]
                gvals = small.tile([16, 2], F32, tag="gvals", name="gvals")
                nc.vector.tensor_copy(gvals[:, 0:1], mall[:, 0:1])
                nc.vector.tensor_copy(gvals[:, 1:2], cur)
                for ct in range(2):
                    cv = ps_gn.tile([128, 2], F32, tag="cv", name="cv")
                    nc.tensor.matmul(
                        cv[:], gbc_sb[:, ct * 128 : (ct + 1) * 128], gvals[:],
                        start=True, stop=True,
                    )
                    scale_t = small.tile([128, 1], F32, tag="scale", name="scale")
                    nc.vector.tensor_tensor(scale_t[:], gnp_sb[ct][:, 0:1], cv[:, 1:2], op=OP.mult)
                    tb = small.tile([128, 1], F32, tag="tb", name="tb")
                    nc.vector.tensor_tensor(tb[:], cv[:, 0:1], scale_t[:], op=OP.mult)
                    bias_t = small.tile([128, 1], F32, tag="bias", name="bias")
                    nc.vector.tensor_tensor(bias_t[:], gnp_sb[ct][:, 1:2], tb[:], op=OP.subtract)
                    for h2 in range(2):
                        nc.vector.tensor_scalar(
                            xn_sb[ct][:, h2 * HALF : (h2 + 1) * HALF],
                            x_sb[ct][:, h2 * HALF : (h2 + 1) * HALF],
                            scale_t[:], bias_t[:],
                            op0=OP.mult, op1=OP.add,
                        )

            # ---- QKV projections ----
            # q/k: [2*HD, HW] channel-major; copies PSUM->SBUF run on ACT
            # (idle here), freeing DVE for GN/stat work.
            q_sb = big.tile([128, HW], BF16, tag="q", name="q")
            k_sb = big.tile([128, HW], BF16, tag="k", name="k")
            # vt: per head, [128 key-partitions, NT pairs x 2 x MPAD] fp8 with a
            # ones column at 64 (softmax denominator accumulator row).
            vt_sb = []
            for h in range(2):
                t = big.tile([128, NJ * MPAD], F8, tag=f"vt{h}", name=f"vt{h}")
                t3 = t[:].rearrange("p (j c) -> p j c", c=MPAD)
                nc.vector.memset(t3[:, :, HD : HD + 1], 1.0)
                nc.vector.memset(t3[:, :, HD + 1 : MPAD], 0.0)
                vt_sb.append(t)

            with tc.tile_pool(name="ps_qkv", bufs=2, space=bass.MemorySpace.PSUM) as ps_qkv:
                for dst, wname in ((q_sb, "wq"), (k_sb, "wk")):
                    for half in range(2):
                        ps = ps_qkv.tile([128, HALF], F32, tag="qkv", name="qkv")
                        for kc in range(2):
                            for n0, n1 in _nchunks(HALF):
                                nc.tensor.matmul(
                                    ps[:, n0:n1],
                                    w_sb[wname, kc][:],
                                    xn_sb[kc][:, half * HALF + n0 : half * HALF + n1],
                                    start=(kc == 0), stop=(kc == 1),
                                )
                        nc.scalar.activation(
                            dst[:, half * HALF : (half + 1) * HALF], ps[:], AF.Copy
                        )
                # v^T: position-major (out partitions = positions) so the fp8
                # vt tiles need no transpose
                for half in range(2):
                    vps = ps_qkv.tile([128, HALF], F32, tag="qkv", name="qkv")
                    for j9 in range(9):
                        jc = half * 9 + j9
                        for kc in range(2):
                            nc.tensor.matmul(
                                vps[:, j9 * 128 : (j9 + 1) * 128],
                                xn_sb[kc][:, jc * JC : (jc + 1) * JC],
                                w_sb["wv", kc][:],
                                start=(kc == 0), stop=(kc == 1),
                            )
                    vps3 = vps[:].rearrange("p (j c) -> p j c", c=128)
                    for h in range(2):
                        dst3 = vt_sb[h][:].rearrange("p (j c) -> p j c", c=MPAD)
                        nc.scalar.activation(
                            dst3[:, half * 9 : (half + 1) * 9, 0:HD],
                            vps3[:, :, h * HD : (h + 1) * HD],
                            AF.Copy,
                        )

            # ---- attention ----
            headout = big.tile([128, HW], BF16, tag="headout", name="headout")
            with (
                tc.tile_pool(name="ps_st", bufs=3, space=bass.MemorySpace.PSUM) as ps_st,
                tc.tile_pool(name="ps_u", bufs=1, space=bass.MemorySpace.PSUM) as ps_u,
            ):
                for i0, i1 in IBLKS:
                    blk = i1 - i0
                    u = [ps_u.tile([MPAD, 512], F32, tag=f"u{h}", name=f"u{h}") for h in range(2)]

                    def emit_s(t):
                        # scores for key chunks (2t, 2t+1), one tile per head;
                        # the two heads' matmuls co-issue on disjoint PE rows
                        sts = []
                        for h in range(2):
                            st = ps_st.tile([128, 1024], F32, tag="st", name="st")
                            sts.append(st)
                        for half in range(2):
                            jc = 2 * t + half
                            for h in range(2):
                                nc.tensor.matmul(
                                    sts[h][:, half * 512 : half * 512 + blk],
                                    k_sb[h * HD : (h + 1) * HD, jc * JC : (jc + 1) * JC],
                                    q_sb[h * HD : (h + 1) * HD, i0:i1],
                                    start=True, stop=True,
                                )
                        return sts

                    def emit_exp(t, sts):
                        # head 0 on ACT (exp -> fp8), head 1 on DVE (schraudolph)
                        pts = []
                        pt0 = ptp.tile([128, 1024], F8, tag="pt0", name="pt0")
                        if blk == 512:
                            nc.scalar.activation(pt0[:], sts[0][:], AF.Exp, scale=1.0 / 16.0)
                        else:
                            for half in range(2):
                                nc.scalar.activation(
                                    pt0[:, half * 512 : half * 512 + blk],
                                    sts[0][:, half * 512 : half * 512 + blk],
                                    AF.Exp, scale=1.0 / 16.0,
                                )
                        pts.append(pt0)
                        pt1 = ptp.tile([128, 1024], I8, tag="pt1", name="pt1")
                        if blk == 512:
                            nc.vector.tensor_scalar(
                                pt1[:], sts[1][:], A8, B8, op0=OP.mult, op1=OP.add
                            )
                        else:
                            for half in range(2):
                                nc.vector.tensor_scalar(
                                    pt1[:, half * 512 : half * 512 + blk],
                                    sts[1][:, half * 512 : half * 512 + blk],
                                    A8, B8, op0=OP.mult, op1=OP.add,
                                )
                        pts.append(pt1[:].bitcast(F8))
                        return pts

                    def emit_pv(t, pts):
                        # fp8 DoubleRow: both key chunks of the pair in one pass
                        for h in range(2):
                            lhsT = vt_sb[h][:, 2 * t * MPAD : (2 * t + 2) * MPAD]
                            lhsT3 = lhsT.rearrange("p (two m) -> p two m", two=2)
                            rhs = pts[h] if h == 0 else pts[h]
                            rhs3 = rhs.rearrange("p (two n) -> p two n", two=2)[:, :, 0:blk]
                            nc.tensor.matmul(
                                u[h][:, 0:blk], lhsT3, rhs3,
                                start=(t == 0), stop=(t == NT - 1),
                                perf_mode=DR,
                            )

                    sts = emit_s(0)
                    for t in range(NT):
                        pts = emit_exp(t, sts)
                        if t + 1 < NT:
                            sts = emit_s(t + 1)
                        emit_pv(t, pts)

                    # ---- normalize + output projection for this block ----
                    for h in range(2):
                        dn = small.tile([1, blk], F32, tag="dn", name="dn")
                        nc.vector.tensor_copy(dn[:], u[h][HD : HD + 1, 0:blk])
                        rcp = small.tile([1, blk], F32, tag="rcp", name="rcp")
                        nc.vector.reciprocal_approx_fast(rcp[:], dn[:])
                        rb = small.tile([HD, blk], F32, tag="rb", name="rb")
                        nc.gpsimd.partition_broadcast(rb[:], rcp[:])
                        nc.vector.tensor_tensor(
                            headout[h * HD : (h + 1) * HD, i0:i1],
                            u[h][0:HD, 0:blk], rb[:], op=OP.mult,
                        )
                    yp = ps_st.tile([128, 1024], F32, tag="st", name="st")
                    for mt in range(2):
                        nc.tensor.matmul(
                            yp[:, mt * 512 : mt * 512 + blk],
                            wo_sb[:, mt * 128 : (mt + 1) * 128],
                            headout[:, i0:i1],
                            start=True, stop=True,
                        )
                        yo = small.tile([128, blk], F32, tag="yo", name="yo")
                        nc.vector.tensor_tensor(
                            yo[:], yp[:, mt * 512 : mt * 512 + blk],
                            res_sb[mt][:, i0:i1], op=OP.add,
                        )
                        nc.sync.dma_start(
                            y_d[mt * 128 : (mt + 1) * 128, i0:i1], yo[:]
                        )

    nc.compile()
    return nc


def _consts():
    # gind[:, 0:16]: tile-0 channel -> group one-hot; [:, 16:32]: tile-1 channel -> group
    gind = np.zeros((128, 32), np.float32)
    for c in range(128):
        gind[c, c // GC] = 1.0
        gind[c, 16 + 8 + c // GC] = 1.0
    gbc = np.zeros((16, C), np.float32)
    for c in range(C):
        gbc[c // GC, c] = 1.0
    return gind, gbc


def make_in_maps(x, gn_weight, gn_bias, qkv_w, out_w, out_b):
    x = np.asarray(x, np.float32)
    qkv_w = np.asarray(qkv_w, np.float32)
    out_w = np.asarray(out_w, np.float32)
    out_b = np.asarray(out_b, np.float32)
    gn_weight = np.asarray(gn_weight, np.float32)
    gn_bias = np.asarray(gn_bias, np.float32)
    xr = np.ascontiguousarray(x.reshape(B, C, HW))
    gind, gbc = _consts()
    gnp = np.ascontiguousarray(np.stack([gn_weight, gn_bias], axis=1))
    in_maps = []
    for core in range(NCORES):
        b, hp = divmod(core, 2)
        heads = (2 * hp, 2 * hp + 1)
        qs = np.concatenate([qkv_w[n * 192 : n * 192 + 64] for n in heads], 0)
        ks = np.concatenate([qkv_w[n * 192 + 64 : n * 192 + 128] for n in heads], 0)
        vs = np.concatenate([qkv_w[n * 192 + 128 : n * 192 + 192] for n in heads], 0)
        res = xr[b] + out_b[:, None] if hp == 0 else np.zeros_like(xr[b])
        in_maps.append({
            "x": xr[b],
            "res": np.ascontiguousarray(res, np.float32),
            "wq": np.ascontiguousarray(qs.T),
            "wk": np.ascontiguousarray(ks.T),
            "wv": np.ascontiguousarray(vs.T),
            "wo": np.ascontiguousarray(out_w[:, hp * 128 : (hp + 1) * 128].T),
            "gnp": gnp,
            "gind": gind,
            "gbc": gbc,
        })
    return in_maps


_NC_CACHE = {}


def get_nc(mm_dt=BF16):
    key = "v2"
    if key not in _NC_CACHE:
        _NC_CACHE[key] = _build()
    return _NC_CACHE[key]


def kernel(x, gn_weight, gn_bias, qkv_w, out_w, out_b):
    nc = get_nc()
    in_maps = make_in_maps(x, gn_weight, gn_bias, qkv_w, out_w, out_b)
    res = bass_utils.run_bass_kernel_spmd(nc, in_maps, core_ids=list(range(NCORES)))
    y = np.empty((B, C, HW), np.float32)
    for b in range(B):
        y[b] = res.results[2 * b]["y"] + res.results[2 * b + 1]["y"]
    return y.reshape(B, C, H, W)
